# revision 24
# baseline (speedup 1.0000x reference)
"""Trainium2 Bass kernel for nn_Aggregation_89575837925422.

Module: feats = [x, dconv3(x), dconv5(x), dconv7(x)] (1920 ch); qk = w_qk@feats;
4-head attention with relative-position biases; out = x + gamma*proj(attn@v).

Sharding across 8 NeuronCores: core c = (batch b=c//4, row-slab s=c%4, 11 rows).

Schedule (v2): three AllGathers, all off the critical path as far as possible.
  t~0   input DMA burst; PE warmup
  t~8   vT for the core's own 484 positions (x_slab^T @ w_v^T) -> AllGather #0
        (completes ~90us, needed ~215us) -- replaces the old per-core full-HW
        vT compute (4x redundant) and the x_full DMA
  t~20  dilated convs; the k = w_qk[6:12]@feats accumulation is interleaved
        into the conv loop (6 PSUM banks hold k blocks; x-part first, then each
        conv mb accumulates one lag behind the conv matmuls), so all 6 k blocks
        finish with the conv and the two k AllGathers trigger immediately
  then  q blocks -> rel-pos tables -> attention pairs (augmented-K matmul folds
        rel-pos into the logit matmul; softmax w/o max-subtraction; denominators
        via a ones column in v^T) -> proj + residual.

All matmul operands bf16 (f32 PSUM accumulation). Output per core: (768, 484).
"""
import os
import sys

sys.path.insert(0, "/opt/trn_rl_repo")

from contextlib import ExitStack

import ml_dtypes
import numpy as np

import concourse.bacc as bacc
import concourse.mybir as mybir
import concourse.tile as tile
from concourse.bass_utils import run_bass_kernel_spmd

dt = mybir.dt
F32 = dt.float32
BF16 = dt.bfloat16
AF = mybir.ActivationFunctionType
BF = ml_dtypes.bfloat16

# ---- problem constants (hardcoded; kernel.py must be self-contained) ----
B = 2
CIN = 768
S = 44
HW = S * S              # 1936
HEAD = 4
D = 192                 # head dim
CO = 384                # conv out channels per dilation
DILS = (3, 5, 7)
NSLAB = 4
SLABR = 11              # rows per slab
NPOS = SLABR * S        # 484
MAXPOS = 100
KC_X = CIN // 128       # 6
C_CONV = 3 * CO         # 1152
MB_CONV = C_CONV // 128  # 9
KC_F = KC_X + MB_CONV   # 15
PADR = 7 + SLABR + 7    # 25
PADW = 64               # 7+44+13 zero pad, 128B row pitch in bf16
NJC = 16                # j-chunks of 121 rows each (16*121 = 1936)
JCH = HW // NJC         # 121
NJC_OWN = NPOS // JCH   # 4 own-slab chunks
VT_W = HEAD * (D + 1)   # 772
EXP_SHIFT = -20.0       # exp(sim - 20); cancels in softmax, avoids fp32 overflow
NCORES = 8
GROUPS = [[0, 1, 2, 3], [4, 5, 6, 7]]


# ---------------------------------------------------------------------------
# host-side input preparation
# ---------------------------------------------------------------------------

def prep_shared(inputs):
    out = {}
    w_conv = np.empty((MB_CONV, 128, 54 * 128), np.float32)
    b_conv = np.empty((128, MB_CONV), np.float32)
    for mb in range(MB_CONV):
        dil_i, mloc = mb // 3, mb % 3
        Wd = np.asarray(inputs[f"w_d{DILS[dil_i]}"], np.float32)
        Wb = Wd[mloc * 128:(mloc + 1) * 128]              # (128m, 768ci, 3, 3)
        t = Wb.reshape(128, KC_X, 128, 3, 3).transpose(3, 4, 1, 2, 0)
        w_conv[mb] = t.reshape(54, 128, 128).transpose(1, 0, 2).reshape(128, 54 * 128)
        b_conv[:, mb] = np.asarray(inputs[f"b_d{DILS[dil_i]}"], np.float32)[mloc * 128:(mloc + 1) * 128]
    out["w_conv"] = w_conv.astype(BF)
    out["b_conv"] = b_conv

    w_qk = np.asarray(inputs["w_qk"], np.float32)         # (1536, 1920)
    qscale = HEAD ** -0.5
    w_qk_l = np.empty((12, 128, KC_F * 128), np.float32)
    for blk in range(12):
        rows = w_qk[blk * 128:(blk + 1) * 128]
        scale = qscale if blk < 6 else 1.0
        w_qk_l[blk] = (rows * scale).reshape(128, KC_F, 128).transpose(2, 1, 0).reshape(
            128, KC_F * 128)
    out["w_qk"] = w_qk_l.astype(BF)

    w_v = np.asarray(inputs["w_v"], np.float32)           # (768, 768) [o, c]
    w_vt = np.zeros((KC_X, 128, VT_W), np.float32)
    for kc in range(KC_X):
        blockT = w_v[:, kc * 128:(kc + 1) * 128].T
        for h in range(HEAD):
            w_vt[kc][:, h * (D + 1):h * (D + 1) + D] = blockT[:, h * D:(h + 1) * D]
    out["w_vt"] = w_vt.astype(BF)

    gamma = float(np.asarray(inputs["gamma"]).reshape(-1)[0])
    w_proj = np.asarray(inputs["w_proj"], np.float32)
    wpa = np.empty((HEAD, 128, CIN), np.float32)
    wpb = np.empty((HEAD, 64, CIN), np.float32)
    for h in range(HEAD):
        wpa[h] = gamma * w_proj[:, h * D:h * D + 128].T
        wpb[h] = gamma * w_proj[:, h * D + 128:(h + 1) * D].T
    out["w_proj_a"] = wpa.astype(BF)
    out["w_proj_b"] = wpb.astype(BF)

    rel_w = np.asarray(inputs["rel_w"], np.float32)
    iy = np.arange(S)
    rw = rel_w[iy[None, :] - iy[:, None] + MAXPOS - 1]    # (y, v, d)
    out["rw"] = np.ascontiguousarray(rw.transpose(2, 0, 1).reshape(D, S * S)).astype(BF)

    j = np.arange(HW)
    U = (j[None, :] // S == np.arange(S)[:, None]).astype(np.float32)
    V = (j[None, :] % S == np.arange(S)[:, None]).astype(np.float32)
    out["k2c"] = np.ascontiguousarray(np.concatenate([U, V[:20]], axis=0)).astype(BF)  # (64, 1936)
    out["k3c"] = np.ascontiguousarray(V[20:44]).astype(BF)                             # (24, 1936)
    return out


def prep_core(inputs, core):
    b, s = core // 4, core % 4
    r0 = s * SLABR
    x = np.asarray(inputs["x"], np.float32)[b]
    out = {}
    xp = np.zeros((KC_X, 128, PADR, PADW), np.float32)
    rlo, rhi = r0 - 7, r0 + SLABR + 7
    glo, ghi = max(rlo, 0), min(rhi, S)
    xr = x.reshape(KC_X, 128, S, S)
    xp[:, :, glo - rlo:ghi - rlo, 7:7 + S] = xr[:, :, glo:ghi, :]
    out["x_pad"] = xp.reshape(KC_X, 128, PADR * PADW).astype(BF)
    xf = xr.reshape(KC_X, 128, HW)
    xs = np.ascontiguousarray(xf[:, :, r0 * S:r0 * S + NPOS])
    out["x_slab_bf"] = xs.astype(BF)
    out["x_res"] = xs
    rel_h = np.asarray(inputs["rel_h"], np.float32)
    ix = np.arange(S)
    rh = rel_h[ix[None, :] - ix[:, None] + MAXPOS - 1]    # (x, u, d)
    out["rh"] = np.ascontiguousarray(
        rh[r0:r0 + SLABR].transpose(2, 0, 1).reshape(D, NPOS)).astype(BF)
    return out


def make_in_maps(inputs):
    shared = prep_shared(inputs)
    in_maps = []
    for c in range(NCORES):
        m = dict(shared)
        m.update(prep_core(inputs, c))
        in_maps.append(m)
    return in_maps


# ---------------------------------------------------------------------------
# Bass program (SPMD, identical on all 8 cores)
# ---------------------------------------------------------------------------

def build_nc():
    nc = bacc.Bacc("TRN2", target_bir_lowering=False, debug=False,
                   num_devices=NCORES)

    d_in = {}
    for name, shape in [
        ("x_pad", [KC_X, 128, PADR * PADW]),
        ("x_slab_bf", [KC_X, 128, NPOS]),
        ("w_conv", [MB_CONV, 128, 54 * 128]),
        ("w_qk", [12, 128, KC_F * 128]),
        ("w_vt", [KC_X, 128, VT_W]),
        ("w_proj_a", [HEAD, 128, CIN]),
        ("w_proj_b", [HEAD, 64, CIN]),
        ("rh", [D, NPOS]),
        ("rw", [D, HW]),
        ("k2c", [64, HW]),
        ("k3c", [24, HW]),
    ]:
        d_in[name] = nc.dram_tensor(name, shape, BF16, kind="ExternalInput")
    d_in["b_conv"] = nc.dram_tensor("b_conv", [128, MB_CONV], F32, kind="ExternalInput")
    d_in["x_res"] = nc.dram_tensor("x_res", [KC_X, 128, NPOS], F32, kind="ExternalInput")
    out_d = nc.dram_tensor("out", [CIN, NPOS], F32, kind="ExternalOutput")

    with tile.TileContext(nc) as tc:
        with ExitStack() as ctx:
            # persistent pools (whole kernel)
            consts = ctx.enter_context(tc.tile_pool(name="consts", bufs=1))
            work = ctx.enter_context(tc.tile_pool(name="work", bufs=2))
            qpool = ctx.enter_context(tc.tile_pool(name="qpool", bufs=1))
            ps2 = ctx.enter_context(tc.tile_pool(name="ps2", bufs=2, space="PSUM"))
            dram = ctx.enter_context(tc.tile_pool(name="dram", bufs=1, space="DRAM"))

            # phase-scoped pools (closed manually, strict LIFO nesting)
            # sQA: small attention q-side tiles, filled piecewise during the
            # q blocks and the rel phase; lives until the end of the kernel
            sQA = ExitStack()
            qapool = sQA.enter_context(tc.tile_pool(name="qapool", bufs=1))
            qa3_t = [qapool.tile([24, NPOS], BF16, tag=f"qa3h{h}", name=f"qa3h{h}")
                     for h in range(HEAD)]
            # head-major q staging, col = g*484 + (x*44 + y).  qhx1 rows 0:128
            # are q d-rows 0:128 (doubles as the sim's qa1 operand via a column
            # slice); qhx2 rows 0:64 are q d-rows 128:192, rows 64:108 rht,
            # rows 108:128 rwt[0:20] (so qhx2[:, gs] is the sim's qa2 operand)
            qhx1 = qapool.tile([128, SLABR * 176], BF16, tag="qhx1", name="qhx1")
            qhx2 = qapool.tile([128, SLABR * 176], BF16, tag="qhx2", name="qhx2")

            sB = ExitStack()   # conv outputs + qk weights + k staging
            convout = sB.enter_context(tc.tile_pool(name="convout", bufs=1))
            qkw = sB.enter_context(tc.tile_pool(name="qkw", bufs=1))
            sA = ExitStack()   # conv inputs/weights (closes after conv+k)
            xpool = sA.enter_context(tc.tile_pool(name="xpool", bufs=1))
            convw = sA.enter_context(tc.tile_pool(name="convw", bufs=22))
            sV = ExitStack()   # vt_own PSUM + staging (closes before conv)
            psV = sV.enter_context(tc.tile_pool(name="psV", bufs=2, space="PSUM"))
            vopool = sV.enter_context(tc.tile_pool(name="vopool", bufs=1))

            # ---- persistent input tiles ----
            # sync queue: x_pad first (first conv matmul needs it)
            xpad_t = []
            for kc in range(KC_X):
                t = xpool.tile([128, PADR * PADW], BF16, tag=f"xpad{kc}")
                nc.sync.dma_start(t[:], d_in["x_pad"].ap()[kc])
                xpad_t.append(t)
            # scalar queue: x_slab + w_vt (vt_own needs them first), bias,
            # then the six k-block qk weights (needed from ~t=15)
            wvt_t = []
            for kc in range(KC_X):
                t = consts.tile([128, VT_W], BF16, tag=f"wvt{kc}")
                nc.scalar.dma_start(t[:], d_in["w_vt"].ap()[kc])
                wvt_t.append(t)
            bconv_t = consts.tile([128, MB_CONV], F32, tag="bconv")
            nc.scalar.dma_start(bconv_t[:], d_in["b_conv"].ap())
            xslab_t = []
            for kc in range(KC_X):
                t = consts.tile([128, NPOS], BF16, tag=f"xslab{kc}")
                nc.scalar.dma_start(t[:], d_in["x_slab_bf"].ap()[kc])
                xslab_t.append(t)
            wqk_t = [qkw.tile([128, KC_F * 128], BF16, tag=f"wqk{b}",
                              name=f"wqk{b}") for b in range(12)]

            # ---- DRAM scratch ----
            ag_in_a = dram.tile([CIN // 2, NPOS], BF16, tag="ag_in_a")
            ag_in_b = dram.tile([CIN // 2, NPOS], BF16, tag="ag_in_b")
            ag_out_a = dram.tile([NSLAB * CIN // 2, NPOS], BF16, tag="ag_out_a")
            ag_out_b = dram.tile([NSLAB * CIN // 2, NPOS], BF16, tag="ag_out_b")
            ag_vt_in = dram.tile([NPOS, VT_W], BF16, tag="ag_vt_in")
            ag_vt_out = dram.tile([NSLAB * NPOS, VT_W], BF16, tag="ag_vt_out")

            # ---- stage 0: PE warmup; ramps the HAM clock while inputs DMA ----
            warm = consts.tile([128, NPOS], BF16, tag="warm")
            nc.vector.memset(warm[:], 0.0)
            for i in range(10):
                wmp = ps2.tile([128, NPOS], F32, tag="mm484", name=f"warm{i}")
                nc.tensor.matmul(wmp[:], warm[:, 0:128], warm[:],
                                 start=True, stop=True)

            def emit_conv_mb(mb, cps):
                first = True
                for kh in range(3):
                    for kw in range(3):
                        t_idx = kh * 3 + kw
                        wt = convw.tile([128, KC_X * 128], BF16, tag="convw")
                        weng = nc.sync if t_idx % 2 == 0 else nc.scalar
                        weng.dma_start(
                            wt[:],
                            d_in["w_conv"].ap()[mb][:, t_idx * KC_X * 128:(t_idx + 1) * KC_X * 128],
                        )
                        dil = DILS[mb // 3]
                        dr, dw = dil * (kh - 1), dil * (kw - 1)
                        for kc in range(KC_X):
                            rhs = xpad_t[kc][:].rearrange(
                                "p (r w) -> p r w", w=PADW
                            )[:, 7 + dr:7 + dr + SLABR, 7 + dw:7 + dw + S]
                            last = (t_idx == 8 and kc == KC_X - 1)
                            nc.tensor.matmul(
                                cps[:], wt[:, kc * 128:(kc + 1) * 128], rhs,
                                start=first, stop=last,
                            )
                            first = False
                co = convout.tile([128, NPOS], BF16, tag=f"conv{mb}")
                nc.scalar.activation(co[:], cps[:], AF.Identity,
                                     bias=bconv_t[:, mb:mb + 1])
                conv_out.append(co)

            # conv mb0 first: it only needs x_pad + one weight tap, so the PE
            # gets real work ~15us earlier than waiting for the vt inputs
            conv_out = []
            sid_conv0 = nc.enter_named_scope("conv", False)[0]
            cps0 = ps2.tile([128, NPOS], F32, tag="mm484", name="cps0")
            emit_conv_mb(0, cps0)
            nc.leave_named_scope("conv", sid_conv0, False)
            for blk in range(6, 12):
                nc.scalar.dma_start(wqk_t[blk][:], d_in["w_qk"].ap()[blk])

            # ---- stage 1: vT for own slab positions -> AllGather #0 ----
            # out[j, hd] = sum_c x[c, j] * w_vT[c, hd]; ones column per head
            # feeds the softmax denominator. Gathered to all 4 group cores.
            sid_vt = nc.enter_named_scope("vt", False)[0]
            for jl in range(NJC_OWN):
                vps = psV.tile([JCH, VT_W], F32, tag="vtps")
                for kc in range(KC_X):
                    lhsT = xslab_t[kc][:, jl * JCH:(jl + 1) * JCH]
                    nc.tensor.matmul(vps[:JCH, 0:512], lhsT, wvt_t[kc][:, 0:512],
                                     start=(kc == 0), stop=(kc == KC_X - 1))
                    nc.tensor.matmul(vps[:JCH, 512:VT_W], lhsT,
                                     wvt_t[kc][:, 512:VT_W],
                                     start=(kc == 0), stop=(kc == KC_X - 1))
                t = vopool.tile([JCH, VT_W], BF16, tag=f"vto{jl}", name=f"vto{jl}")
                nc.vector.tensor_copy(t[:], vps[:JCH])
                for h in range(HEAD):
                    col = h * (D + 1) + D
                    nc.vector.memset(t[:, col:col + 1], 1.0)
                # gpsimd DMA trigger: before the first collective trigger on
                # that queue, so nothing blocks; keeps sync/scalar free for
                # the conv weight stream
                nc.gpsimd.dma_start(ag_vt_in[jl * JCH:(jl + 1) * JCH, :], t[:])
            sid_agv = nc.enter_named_scope("agvt", False)[0]
            nc.gpsimd.collective_compute(
                "AllGather", mybir.AluOpType.bypass,
                ins=[ag_vt_in[:]], outs=[ag_vt_out[:]], replica_groups=GROUPS,
            )
            nc.leave_named_scope("agvt", sid_agv, False)
            nc.leave_named_scope("vt", sid_vt, False)
            sV.close()

            # ---- stage 2: k x-part, then conv with interleaved k accumulation
            sK = ExitStack()
            psK = sK.enter_context(tc.tile_pool(name="psK", bufs=1, space="PSUM"))
            kps = [psK.tile([128, NPOS], F32, tag=f"kps{i}", name=f"kps{i}")
                   for i in range(6)]
            sid_kx = nc.enter_named_scope("kx", False)[0]
            for i in range(6):
                for kc in range(KC_X):
                    nc.tensor.matmul(kps[i][:],
                                     wqk_t[6 + i][:, kc * 128:(kc + 1) * 128],
                                     xslab_t[kc][:],
                                     start=(kc == 0), stop=False)
            nc.leave_named_scope("kx", sid_kx, False)

            # q-block weights + rel tables stream during the conv, keeping the
            # k-AllGather window free of competing HBM traffic
            for blk in range(6):
                nc.scalar.dma_start(wqk_t[blk][:], d_in["w_qk"].ap()[blk])
            rh1 = consts.tile([128, NPOS], BF16, tag="rh1")
            nc.scalar.dma_start(rh1[:], d_in["rh"].ap()[0:128, :])
            rh2 = consts.tile([64, NPOS], BF16, tag="rh2")
            nc.scalar.dma_start(rh2[:], d_in["rh"].ap()[128:D, :])
            rw1 = consts.tile([128, HW], BF16, tag="rw1")
            nc.scalar.dma_start(rw1[:], d_in["rw"].ap()[0:128, :])
            rw2 = consts.tile([64, HW], BF16, tag="rw2")
            nc.scalar.dma_start(rw2[:], d_in["rw"].ap()[128:D, :])

            sid_conv = nc.enter_named_scope("conv", False)[0]

            def k_accum(mb):
                # one conv-mb of the k contraction; lags conv by one mb so the
                # PSUM->SBUF bias activation never stalls the PE queue
                kc = KC_X + mb
                for i in range(6):
                    nc.tensor.matmul(kps[i][:],
                                     wqk_t[6 + i][:, kc * 128:(kc + 1) * 128],
                                     conv_out[mb][:],
                                     start=False, stop=(mb == MB_CONV - 1))

            for mb in range(1, MB_CONV):
                cps = ps2.tile([128, NPOS], F32, tag="mm484")
                emit_conv_mb(mb, cps)
                k_accum(mb - 1)
            k_accum(MB_CONV - 1)
            nc.leave_named_scope("conv", sid_conv, False)

            # ---- stage 3: k copies -> DRAM -> the two k AllGathers ----
            sid_qk = nc.enter_named_scope("qk_k", False)[0]
            ksb_t = []
            for i in range(6):
                sb = qkw.tile([128, NPOS], BF16, tag=f"ksb{i}", name=f"ksb{i}")
                nc.vector.tensor_copy(sb[:], kps[i][:])
                ksb_t.append(sb)
                dst = ag_in_a if i < 3 else ag_in_b
                nc.scalar.dma_start(dst[(i % 3) * 128:(i % 3) * 128 + 128, :],
                                    sb[:])
                if i == 2:
                    sid_ag = nc.enter_named_scope("allgather", False)[0]
                    nc.gpsimd.collective_compute(
                        "AllGather", mybir.AluOpType.bypass,
                        ins=[ag_in_a[:]], outs=[ag_out_a[:]],
                        replica_groups=GROUPS,
                    )
                    nc.leave_named_scope("allgather", sid_ag, False)
                if i == 5:
                    sid_ag2 = nc.enter_named_scope("allgather2", False)[0]
                    nc.gpsimd.collective_compute(
                        "AllGather", mybir.AluOpType.bypass,
                        ins=[ag_in_b[:]], outs=[ag_out_b[:]],
                        replica_groups=GROUPS,
                    )
                    nc.leave_named_scope("allgather2", sid_ag2, False)
            nc.leave_named_scope("qk_k", sid_qk, False)
            sK.close()
            sA.close()

            def q_row_splits(r0g, n):
                """Split q global rows [r0g, r0g+n) into (tile, src_off, dst_off, n)."""
                parts = []
                done = 0
                while done < n:
                    t_i, off = divmod(r0g + done, 128)
                    take = min(128 - off, n - done)
                    parts.append((t_i, off, done, take))
                    done += take
                return parts

            def stage_q_pieces(b):
                # every attention-side copy whose source is q block b: the rel
                # gather layout (qhx1/qhx2) and the per-head qa1 / qa2 d-rows
                for g in range(HEAD):
                    gs = slice(g * NPOS, (g + 1) * NPOS)
                    for (t_i, off, d0, take) in q_row_splits(g * D, 128):
                        if t_i == b:
                            nc.scalar.dma_start(qhx1[d0:d0 + take, gs],
                                                q_sb[b][off:off + take, :])
                    for (t_i, off, d0, take) in q_row_splits(g * D + 128, 64):
                        if t_i == b:
                            nc.scalar.dma_start(qhx2[d0:d0 + take, gs],
                                                q_sb[b][off:off + take, :])

            # ---- stage 4: q blocks (into SBUF) ----
            q_sb = [qpool.tile([128, NPOS], BF16, tag=f"qsb{b}", name=f"qsb{b}")
                    for b in range(KC_X)]

            def feats_rhs(kc):
                if kc < KC_X:
                    return xslab_t[kc][:]
                return conv_out[kc - KC_X][:]

            sid_qq = nc.enter_named_scope("qk_q", False)[0]
            for blk in range(6):
                qps = ps2.tile([128, NPOS], F32, tag="mm484", name=f"qps{blk}")
                for kc in range(KC_F):
                    nc.tensor.matmul(qps[:], wqk_t[blk][:, kc * 128:(kc + 1) * 128],
                                     feats_rhs(kc),
                                     start=(kc == 0), stop=(kc == KC_F - 1))
                nc.vector.tensor_copy(q_sb[blk][:], qps[:])
                stage_q_pieces(blk)
            nc.leave_named_scope("qk_q", sid_qq, False)
            sB.close()

            # big attention tiles + attention PSUM pools (after conv SBUF freed)
            sE = ExitStack()
            augpool = sE.enter_context(tc.tile_pool(name="augpool", bufs=4))
            aug1pool = sE.enter_context(tc.tile_pool(name="aug1pool", bufs=1))
            exppool = sE.enter_context(tc.tile_pool(name="exppool", bufs=12))
            psE = sE.enter_context(tc.tile_pool(name="psE", bufs=1, space="PSUM"))
            psS = sE.enter_context(tc.tile_pool(name="psS", bufs=2, space="PSUM"))

            # gathered vT tiles for the AV matmuls (AllGather #0 long done);
            # allocated only now so the conv phase has SBUF headroom for a
            # deep conv-weight prefetch buffer
            # tile_wait_until: the tile scheduler does not model collective
            # latency, so without the hint it parks these gather-dependent
            # loads early in the DMA rings where they dam everything behind
            # them (head-of-line blocking observed as a 36us conv stall)
            vt_t = []
            with tc.tile_wait_until(0.135):
                for jc in range(NJC):
                    t = augpool.tile([JCH, VT_W], BF16, tag=f"vt{jc}",
                                     name=f"vt{jc}", bufs=1)
                    nc.sync.dma_start(t[:], ag_vt_out[jc * JCH:(jc + 1) * JCH, :])
                    vt_t.append(t)

            # proj weights + attn output tiles (persist to the end)
            sPJ = ExitStack()
            projpool = sPJ.enter_context(tc.tile_pool(name="projpool", bufs=1))
            wpa_t, wpb_t = [], []
            with tc.tile_wait_until(0.26):
                for h in range(HEAD):
                    ta = projpool.tile([128, CIN], BF16, tag=f"wpa{h}",
                                       name=f"wpa{h}")
                    nc.sync.dma_start(ta[:], d_in["w_proj_a"].ap()[h])
                    wpa_t.append(ta)
                    tb = projpool.tile([64, CIN], BF16, tag=f"wpb{h}",
                                       name=f"wpb{h}")
                    nc.sync.dma_start(tb[:], d_in["w_proj_b"].ap()[h])
                    wpb_t.append(tb)
            # attention outputs, written in place by the normalize multiply
            at_a = [projpool.tile([128, NPOS], BF16, tag=f"ata{h}", name=f"ata{h}")
                    for h in range(HEAD)]
            at_b = [projpool.tile([64, NPOS], BF16, tag=f"atb{h}", name=f"atb{h}")
                    for h in range(HEAD)]
            # residual x (f32) for the final add
            xres_t = []
            with tc.tile_wait_until(0.26):
                for kc in range(KC_X):
                    t = projpool.tile([128, NPOS], F32, tag=f"xres{kc}",
                                      name=f"xres{kc}")
                    nc.sync.dma_start(t[:], d_in["x_res"].ap()[kc])
                    xres_t.append(t)
            # f32 partial proj accumulators (pair0's half of the contraction)
            pjpart_t = [projpool.tile([128, NPOS], F32, tag=f"pjp{mb}", name=f"pjp{mb}")
                        for mb in range(KC_X)]

            # attention constants
            shift_t = consts.tile([128, 1], F32, tag="shift")
            nc.vector.memset(shift_t[:], EXP_SHIFT)
            ka3 = aug1pool.tile([24, HW], BF16, tag="ka3")
            nc.sync.dma_start(ka3[:], d_in["k3c"].ap())

            def emit_rel():
                sid = nc.enter_named_scope("rel", False)[0]
                # q was gathered piecewise during the q blocks into qhx1/qhx2
                # in head-major layout: col = g*484 + a*44 + b (so the per-block
                # staging copies are contiguous); the matmuls below use strided
                # views to pick fixed-a / fixed-b slices
                qv1 = qhx1[:].rearrange("p (g a b) -> p g a b", g=HEAD, b=S)
                qv2 = qhx2[0:64, :].rearrange("p (g a b) -> p g a b", g=HEAD, b=S)

                # rht[u, (x, g, y)]: one matmul pair per slab row x
                rht_all = qapool.tile([S, SLABR * 176], BF16, tag="rht_all",
                                       name="rhta")
                for xl in range(SLABR):
                    rps = ps2.tile([S, 176], F32, tag="mm484", name=f"rhtp{xl}")
                    cs = slice(xl * S, (xl + 1) * S)
                    nc.tensor.matmul(rps[0:S, :], rh1[:, cs],
                                     qv1[:, :, xl, :],
                                     start=True, stop=False)
                    nc.tensor.matmul(rps[0:S, :], rh2[:, cs],
                                     qv2[:, :, xl, :],
                                     start=False, stop=True)
                    nc.vector.tensor_copy(rht_all[:, xl * 176:(xl + 1) * 176],
                                          rps[0:S, :])

                # rwt[v, (y, g, x)]: one matmul pair per column y
                rwt_all = qapool.tile([S, S * 44], BF16, tag="rwt_all", name="rwta")
                for y in range(S):
                    wps = ps2.tile([S, 44], F32, tag="mm484", name=f"rwtp{y}")
                    cs = slice(y * S, (y + 1) * S)
                    nc.tensor.matmul(wps[0:S, :], rw1[:, cs], qv1[:, :, :, y],
                                     start=True, stop=False)
                    nc.tensor.matmul(wps[0:S, :], rw2[:, cs], qv2[:, :, :, y],
                                     start=False, stop=True)
                    nc.vector.tensor_copy(rwt_all[:, y * 44:(y + 1) * 44],
                                          wps[0:S, :])

                rhv = rht_all[:].rearrange("p (a g b) -> p a g b", g=HEAD, b=S)
                rwv = rwt_all[:].rearrange("p (b g a) -> p b g a", g=HEAD, a=SLABR)
                for h in range(HEAD):
                    rsb = qapool.tile([S, NPOS], BF16, tag=f"rhtsb{h}",
                                      name=f"rhtsb{h}")
                    nc.vector.tensor_copy(
                        rsb[:].rearrange("p (a b) -> p a b", b=S), rhv[:, :, h, :])
                    wsb = qapool.tile([S, NPOS], BF16, tag=f"rwtsb{h}",
                                      name=f"rwtsb{h}")
                    nc.vector.tensor_copy(
                        wsb[:].rearrange("p (a b) -> p a b", b=S),
                        rwv[:, :, h, :].rearrange("p b a -> p a b"))
                    # straight into the augmented-q staging (partition shift via DMA)
                    hs = slice(h * NPOS, (h + 1) * NPOS)
                    nc.scalar.dma_start(qhx2[64:108, hs], rsb[:])
                    nc.scalar.dma_start(qhx2[108:128, hs], wsb[0:20, :])
                    nc.scalar.dma_start(qa3_t[h][:], wsb[20:44, :])
                nc.leave_named_scope("rel", sid, False)

            # ---- stage 5: attention, two heads interleaved per pass ----
            def build_k_tiles(h):
                ka1 = augpool.tile([128, HW], BF16, tag="ka1", name=f"ka1h{h}")
                ka2 = augpool.tile([128, HW], BF16, tag="ka2", name=f"ka2h{h}")
                ago = ag_out_a if h < 2 else ag_out_b
                hh = h % 2
                srcv = ago[:].rearrange("(sl c) q -> c sl q", c=CIN // 2)
                with tc.tile_wait_until(0.21 if h < 2 else 0.24):
                    nc.sync.dma_start(
                        ka1[:].rearrange("p (sl q) -> p sl q", q=NPOS),
                        srcv[hh * D:hh * D + 128])
                    nc.scalar.dma_start(
                        ka2[0:64, :].rearrange("p (sl q) -> p sl q", q=NPOS),
                        srcv[hh * D + 128:hh * D + D])
                    nc.sync.dma_start(ka2[64:128, :], d_in["k2c"].ap())
                return ka1, ka2

            def emit_av(av, item):
                h, idx, jc, ex = item
                jn = JCH
                av1, av2 = av[h]
                vt = vt_t[jc]
                c0 = h * (D + 1)
                nc.tensor.matmul(av1[:], vt[0:jn, c0:c0 + 128], ex[:jn],
                                 start=(idx == 0), stop=(idx == NJC - 1))
                nc.tensor.matmul(av2[:], vt[0:jn, c0 + 128:c0 + 193], ex[:jn],
                                 start=(idx == 0), stop=(idx == NJC - 1))

            def emit_proj_half(half, heads):
                sid_pj = nc.enter_named_scope(f"proj{half}", False)[0]
                for mb in range(KC_X):
                    pps = ps2.tile([128, NPOS], F32, tag="mm484",
                                   name=f"pj{half}_{mb}")
                    ops = []
                    for h in heads:
                        ops.append((wpa_t[h], at_a[h], 128))
                        ops.append((wpb_t[h], at_b[h], 64))
                    for i, (w, a, pr) in enumerate(ops):
                        nc.tensor.matmul(pps[:], w[0:pr, mb * 128:(mb + 1) * 128],
                                         a[0:pr, :],
                                         start=(i == 0), stop=(i == len(ops) - 1))
                    if half == 0:
                        # fold the residual in here, off the tail's critical path
                        nc.vector.tensor_add(pjpart_t[mb][:], pps[:], xres_t[mb][:])
                    else:
                        ot = work.tile([128, NPOS], F32, tag="outsb", bufs=3)
                        nc.vector.tensor_add(ot[:], pps[:], pjpart_t[mb][:])
                        nc.sync.dma_start(out_d.ap()[mb * 128:(mb + 1) * 128, :],
                                          ot[:])
                nc.leave_named_scope(f"proj{half}", sid_pj, False)

            # prefetch pair0's K tiles while rel computes
            ka_tiles = {}
            for h in (0, 1):
                ka_tiles[h] = build_k_tiles(h)
            emit_rel()

            av_all = {}

            def make_av(h):
                av_all[h] = (
                    psE.tile([128, NPOS], F32, tag=f"av1_{h % 2}", name=f"av1h{h}"),
                    psE.tile([65, NPOS], F32, tag=f"av2_{h % 2}", name=f"av2h{h}"))

            def sim_exp(h, jc, pend, drain=True):
                ka1, ka2 = ka_tiles[h]
                j0 = jc * JCH
                jn = JCH
                hs = slice(h * NPOS, (h + 1) * NPOS)
                sps = psS.tile([JCH, NPOS], F32, tag="simps")
                nc.tensor.matmul(sps[:jn], ka1[:, j0:j0 + jn], qhx1[:, hs],
                                 start=True, stop=False)
                nc.tensor.matmul(sps[:jn], ka2[:, j0:j0 + jn], qhx2[:, hs],
                                 start=False, stop=False)
                nc.tensor.matmul(sps[:jn], ka3[:, j0:j0 + jn], qa3_t[h][:],
                                 start=False, stop=True)
                ex = exppool.tile([JCH, NPOS], BF16, tag="expt")
                nc.scalar.activation(ex[:jn], sps[:jn], AF.Exp,
                                     bias=shift_t[:jn, :])
                pend.append((h, jc, jc, ex))
                if drain and len(pend) > 2:
                    emit_av(av_all, pend.pop(0))

            def normalize(heads):
                for h in heads:
                    av1, av2 = av_all[h]
                    den = work.tile([65, NPOS], F32, tag="densb", bufs=2)
                    nc.vector.reciprocal(den[64:65, :], av2[64:65, :])
                    rrec = work.tile([1, NPOS], F32, tag="rrec", bufs=2)
                    nc.sync.dma_start(rrec[:], den[64:65, :])
                    rall = work.tile([128, NPOS], F32, tag="rall", bufs=2)
                    nc.gpsimd.partition_broadcast(rall[:], rrec[:])
                    nc.vector.tensor_tensor(at_a[h][:], av1[:], rall[:],
                                            mybir.AluOpType.mult)
                    nc.vector.tensor_tensor(at_b[h][:], av2[0:64, :],
                                            rall[0:64, :],
                                            mybir.AluOpType.mult)

            # ---- pair 0: heads 0,1 over all 16 chunks ----
            sid_h = nc.enter_named_scope("pair0", False)[0]
            make_av(0)
            make_av(1)
            pend0 = []
            for jc in range(NJC):
                for h in (0, 1):
                    sim_exp(h, jc, pend0)
            for h in (2, 3):
                ka_tiles[h] = build_k_tiles(h)
            while pend0:
                emit_av(av_all, pend0.pop(0))
            normalize((0, 1))
            nc.leave_named_scope("pair0", sid_h, False)

            # ---- pair 1 sims keep the PE warm while heads 0,1 normalize;
            # proj half 0 slots in once its inputs are ready ----
            sid_h = nc.enter_named_scope("pair1", False)[0]
            make_av(2)
            make_av(3)
            pend1 = []
            for jc in range(4):
                for h in (2, 3):
                    sim_exp(h, jc, pend1, drain=False)
            nc.leave_named_scope("pair1", sid_h, False)
            emit_proj_half(0, (0, 1))
            sid_h = nc.enter_named_scope("pair1b", False)[0]
            for jc in range(4, NJC):
                for h in (2, 3):
                    sim_exp(h, jc, pend1)
            while pend1:
                emit_av(av_all, pend1.pop(0))
            normalize((2, 3))
            nc.leave_named_scope("pair1b", sid_h, False)
            emit_proj_half(1, (2, 3))
            sPJ.close()
            sE.close()
            sQA.close()

    nc.compile()
    return nc


_NC_CACHE = None
last_exec_time_ns = None
last_results = None


def kernel(**inputs):
    global _NC_CACHE, last_exec_time_ns, last_results
    if _NC_CACHE is None:
        _NC_CACHE = build_nc()
    in_maps = make_in_maps(inputs)
    trace = bool(int(os.environ.get("AGG_TRACE", "0")))
    res = run_bass_kernel_spmd(_NC_CACHE, in_maps, list(range(NCORES)), trace=trace)
    last_exec_time_ns = res.exec_time_ns
    last_results = res
    final = np.empty((B, CIN, S, S), np.float32)
    for c in range(NCORES):
        b, s = c // 4, c % 4
        final[b, :, s * SLABR:(s + 1) * SLABR, :] = (
            res.results[c]["out"].reshape(CIN, SLABR, S))
    return final
